# revision 25
# baseline (speedup 1.0000x reference)
"""Trainium2 Bass kernel for nn_AttentionModel (B=4,S=2048,H=8,E=64, dropout mask).

Sharding: 32 (b,h) pairs over 8 cores (4 pairs/core). Device computes, per
(pair, s-chunk-of-1024) unit, transposed-score attention with ALL main-loop
matmuls in one 64x64 PE-tiling config so the four quadrant tiles can run
concurrently (no mode-switch drains):

  step u (= t-rows 128u..128u+128 of one s-chunk of 1024):
    scores: 4 quadrant MMs K=64(e) M=64(t) N=512 -> sp[128,1024] F32 psum
            bank0 (s 0:512)   <- row-0 tiles (0,0)+(0,64)
            bank1 (s 512:1024)<- row-64 tiles (64,0)+(64,64)  [q/k dup'd]
    exp:    one ACT instr [128,1024] (scores pre-scaled by 1/8 on host)
    mask:   DMA [128,1024] fp16; pr = ex*mk on DVE (fp16 2x mode)
    PV/den: per s-half, 4 concurrent quadrant MMs:
            PV-even (0,0) -> pvA[0:64], den-lo (0,64) ones[64,64] -> pvA[64:128]
            PV-odd (64,64) -> pvB[64:128], den-hi (64,0) -> pvB[0:64]
  finalize: DVE copy psum->SBUF, DMA unnormalized PV + den rows to DRAM.

Host does the QKV projections (BLAS), all transposes/dup-layout prep, and the
final (pvA+pvB)/(0.9*den) normalization + gather.
"""

import sys

sys.path.insert(0, "/opt/trn_rl_repo")

import numpy as np

import concourse.bass as bass
import concourse.mybir as mybir
import concourse.tile as tile
from concourse import bacc, bass_utils
from concourse.bass import ds, ts

B, S, H, E = 4, 2048, 8, 64
NCORES = 8
PAIRS = (B * H) // NCORES  # 4 pairs per core
SC = 1024                  # s-chunk width
NSC = S // SC              # 2
NTT = S // 128             # 16 t-tiles (steps) per unit
DEPTH = 4                  # PV trails scores by DEPTH steps
MPF = 6                    # mask DMA prefetch distance (steps)
F32 = mybir.dt.float32
FP16 = mybir.dt.float16
KEEP = 0.9

_CACHED_NC = None


def _body(tc, qpd_d, kpd_d, vpd_d, mT_d, outA_d, outB_d):
    nc = tc.nc
    Exp = mybir.ActivationFunctionType.Exp
    with (
        tc.tile_pool(name="const", bufs=1) as const,
        tc.tile_pool(name="io", bufs=2) as io,
        tc.tile_pool(name="mk", bufs=MPF + 2) as mkp,
        tc.tile_pool(name="work", bufs=11) as work,
        tc.tile_pool(name="fin", bufs=2) as fin,
        tc.tile_pool(name="psS", bufs=2, space=bass.MemorySpace.PSUM) as psS,
        tc.tile_pool(name="psA", bufs=1, space=bass.MemorySpace.PSUM) as psA,
        tc.tile_pool(name="psB", bufs=1, space=bass.MemorySpace.PSUM) as psB,
    ):
        onesw = const.tile([128, 64], FP16, tag="onesw")
        nc.vector.memset(onesw[:, :], 1.0)

        # per-pair input tiles (double-buffered across pairs)
        def load_pair(p, eng):
            qpd = io.tile([128, S], FP16, tag="qpd", name="qpd")
            kpd = io.tile([128, S], FP16, tag="kpd", name="kpd")
            vpd = io.tile([128, NTT * E], FP16, tag="vpd", name="vpd")
            eng.dma_start(qpd[:, :], qpd_d[p])
            eng.dma_start(kpd[:, :], kpd_d[p])
            eng.dma_start(vpd[:, :], vpd_d[p])
            return qpd, kpd, vpd

        # SWDGE warmup: pay the Q7 first-use cost off the critical path
        warm = const.tile([128, 1], FP16, tag="warm")
        nc.gpsimd.dma_start(warm[:, :], qpd_d[0, :, 0:1])
        pair_tiles = {}

        units = [(p, c) for p in range(PAIRS) for c in range(NSC)]
        N = len(units) * NTT  # 128 steps
        exs, prs, pvts, mks = {}, {}, {}, {}

        def load_mask(gj, eng=None):
            unit, u = divmod(gj, NTT)
            p, c = units[unit]
            mk = mkp.tile([128, SC], FP16, tag="mk", name="mk")
            (eng or nc.sync).dma_start(
                mk[:, :], mT_d[p, ds(128 * u, 128), ds(c * SC, SC)])
            mks[gj] = mk

        def scores_step(gj):
            unit, u = divmod(gj, NTT)
            p, c = units[unit]
            if c == 0 and u == 0 and p + 1 < PAIRS:
                pair_tiles[p + 1] = load_pair(p + 1, nc.gpsimd)
            if gj + MPF < N:
                load_mask(gj + MPF)
            qpd, kpd, vpd = pair_tiles[p]
            sp = psS.tile([128, SC], F32, tag="sp", name="sp")
            t0 = 128 * u
            # 4 concurrent quadrant MMs; row-0 tiles -> bank0, row-64 -> bank1
            nc.tensor.matmul(sp[0:64, 0:512], kpd[0:64, ds(t0, 64)],
                             qpd[0:64, ds(c * SC, 512)],
                             start=True, stop=True, tile_position=(0, 0))
            nc.tensor.matmul(sp[64:128, 0:512], kpd[0:64, ds(t0 + 64, 64)],
                             qpd[0:64, ds(c * SC, 512)],
                             start=True, stop=True, tile_position=(0, 64))
            nc.tensor.matmul(sp[0:64, 512:1024], kpd[64:128, ds(t0, 64)],
                             qpd[64:128, ds(c * SC + 512, 512)],
                             start=True, stop=True, tile_position=(64, 0))
            nc.tensor.matmul(sp[64:128, 512:1024],
                             kpd[64:128, ds(t0 + 64, 64)],
                             qpd[64:128, ds(c * SC + 512, 512)],
                             start=True, stop=True, tile_position=(64, 64))
            ex = work.tile([128, SC], FP16, tag="ex", name="ex")
            nc.scalar.activation(ex[:, :], sp[:, :], Exp)
            exs[gj] = ex

        def mul_step(gj):
            ex = exs[gj]
            mk = mks.pop(gj)
            pr = work.tile([128, SC], FP16, tag="pr", name="pr")
            nc.vector.tensor_mul(pr[:, :], ex[:, :], mk[:, :])
            prs[gj] = pr

        def pv_step(gj):
            unit, u = divmod(gj, NTT)
            p, c = units[unit]
            _, _, vpd = pair_tiles[p]
            ex, pr = exs.pop(gj), prs.pop(gj)
            if u == 0:
                pvA = psA.tile([128, SC], F32, tag="pvA", name="pvA")
                pvB = psB.tile([128, SC], F32, tag="pvB", name="pvB")
                pvts[unit] = (pvA, pvB)
            pvA, pvB = pvts[unit]
            st = (u == 0)
            sp_ = (u == NTT - 1)
            vsl = ts(u, E)
            for s in range(2):      # s-half (512 cols)
                o = ds(s * 512, 512)
                # PV first: its start=True clears the bank before the
                # start=False den MMs write into it
                nc.tensor.matmul(pvA[0:64, o], vpd[0:64, vsl],
                                 pr[0:64, o], start=st, stop=sp_,
                                 tile_position=(0, 0))
                nc.tensor.matmul(pvB[64:128, o], vpd[64:128, vsl],
                                 pr[64:128, o], start=st, stop=sp_,
                                 tile_position=(64, 64))
                nc.tensor.matmul(pvA[64:128, o], onesw[0:64, :],
                                 ex[0:64, o], start=st, stop=sp_,
                                 tile_position=(0, 64))
                nc.tensor.matmul(pvB[0:64, o], onesw[64:128, :],
                                 ex[64:128, o], start=st, stop=sp_,
                                 tile_position=(64, 0))

        def finalize(unit):
            p, c = units[unit]
            pvA, pvB = pvts.pop(unit)
            obA = fin.tile([128, SC], F32, tag="obA", name="obA")
            nc.vector.tensor_copy(obA[:, :], pvA[:, :])
            obB = fin.tile([128, SC], F32, tag="obB", name="obB")
            nc.vector.tensor_copy(obB[:, :], pvB[:, :])
            eng = nc.sync if unit == len(units) - 1 else nc.gpsimd
            eng.dma_start(outA_d[p, c, 0:64], obA[0:64, :])
            eng.dma_start(outA_d[p, c, 64:65], obA[64:65, :])
            eng.dma_start(outB_d[p, c, 0:1], obB[0:1, :])
            eng.dma_start(outB_d[p, c, 1:65], obB[64:128, :])

        # scores(idx) issued first each iteration (keeps ACT fed), PV at a
        # uniform lag of DEPTH, finalize immediately after a unit's last PV
        # (its evac copies enter the DVE queue ahead of the next mul).
        # pv(gj) at uniform lag DEPTH, except a unit's LAST pv step is
        # co-issued one iteration early (with its second-to-last), so the
        # evac gets 2 iterations of runway before the next unit's PV chain
        # needs the psum banks.
        # prologue: q/k for pair 0 first (scores(0) needs them), then the
        # first masks, then v (not needed until the first pv step)
        qpd0 = io.tile([128, S], FP16, tag="qpd", name="qpd")
        kpd0 = io.tile([128, S], FP16, tag="kpd", name="kpd")
        vpd0 = io.tile([128, NTT * E], FP16, tag="vpd", name="vpd")
        # first-needed slices first: scores(0..7) read kpd cols 0:1024 and
        # qpd cols 0:1024 (unit 0 is s-chunk 0)
        nc.sync.dma_start(kpd0[:, 0:SC], kpd_d[0, :, 0:SC])
        nc.sync.dma_start(qpd0[:, 0:SC], qpd_d[0, :, 0:SC])
        for g in range(MPF):
            load_mask(g, nc.gpsimd)
        nc.sync.dma_start(kpd0[:, SC:S], kpd_d[0, :, SC:S])
        nc.sync.dma_start(qpd0[:, SC:S], qpd_d[0, :, SC:S])
        nc.sync.dma_start(vpd0[:, :], vpd_d[0])
        pair_tiles[0] = (qpd0, kpd0, vpd0)
        for idx in range(N + DEPTH):
            gj = idx - DEPTH
            boundary = gj >= 0 and (gj + 2) % NTT == NTT - 1
            if idx < N:
                scores_step(idx)
                if not boundary:
                    mul_step(idx)
            if gj >= 0 and gj % NTT < NTT - 2:
                pv_step(gj)
                if boundary:
                    pv_step(gj + 1)      # last two pv steps of the unit,
                    pv_step(gj + 2)      # co-issued two iterations early
                    finalize(gj // NTT)
                    if idx < N:
                        mul_step(idx)


def _build():
    global _CACHED_NC
    if _CACHED_NC is not None:
        return _CACHED_NC
    nc = bacc.Bacc("TRN2", target_bir_lowering=False, debug=False,
                   num_devices=NCORES)
    qpd_d = nc.dram_tensor("qpd", [PAIRS, 128, S], FP16,
                           kind="ExternalInput").ap()
    kpd_d = nc.dram_tensor("kpd", [PAIRS, 128, S], FP16,
                           kind="ExternalInput").ap()
    vpd_d = nc.dram_tensor("vpd", [PAIRS, 128, NTT * E], FP16,
                           kind="ExternalInput").ap()
    mT_d = nc.dram_tensor("maskT", [PAIRS, S, S], FP16,
                          kind="ExternalInput").ap()
    outA_d = nc.dram_tensor("outA", [PAIRS, NSC, 65, SC], F32,
                            kind="ExternalOutput").ap()
    outB_d = nc.dram_tensor("outB", [PAIRS, NSC, 65, SC], F32,
                            kind="ExternalOutput").ap()
    with tile.TileContext(nc) as tc:
        _body(tc, qpd_d, kpd_d, vpd_d, mT_d, outA_d, outB_d)
    nc.compile()
    _CACHED_NC = nc
    return nc


def _in_maps(inputs):
    f32 = np.float32
    query = np.asarray(inputs["query"], f32)
    key = np.asarray(inputs["key"], f32)
    value = np.asarray(inputs["value"], f32)
    mask = np.asarray(inputs["drop_mask"])
    Wq, bq = np.asarray(inputs["Wq"], f32), np.asarray(inputs["bq"], f32)
    Wk, bk = np.asarray(inputs["Wk"], f32), np.asarray(inputs["bk"], f32)
    Wv, bv = np.asarray(inputs["Wv"], f32), np.asarray(inputs["bv"], f32)

    # host-side projections (BLAS) -- [B,S,H,E] @ [E,E] + b
    qp = (query.reshape(-1, E) @ Wq + bq).reshape(B, S, H, E)
    kp = (key.reshape(-1, E) @ Wk + bk).reshape(B, S, H, E)
    vp = (value.reshape(-1, E) @ Wv + bv).reshape(B, S, H, E)

    # qpd/kpd: [BH, E, S] fp16, duplicated across partition halves -> [BH,128,S]
    qpT = (qp.transpose(0, 2, 3, 1).reshape(B * H, E, S) * (1.0 / 8.0))
    kpT = kp.transpose(0, 2, 3, 1).reshape(B * H, E, S)
    qpd = np.concatenate([qpT, qpT], axis=1).astype(np.float16)
    kpd = np.concatenate([kpT, kpT], axis=1).astype(np.float16)
    # vpd: [BH, 128, 16*E]: partition p, block u holds v'[t=128u+p, :]
    vpd = (vp.transpose(0, 2, 1, 3).reshape(B * H, NTT, 128, E)
           .transpose(0, 2, 1, 3).reshape(B * H, 128, NTT * E)
           .astype(np.float16))
    # mask transposed [BH, t, s] as fp16 {0,1}
    mT = (np.ascontiguousarray(mask.transpose(0, 1, 3, 2))
          .astype(np.float16).reshape(B * H, S, S))

    maps = []
    for cidx in range(NCORES):
        sl = slice(cidx * PAIRS, (cidx + 1) * PAIRS)
        maps.append({
            "qpd": np.ascontiguousarray(qpd[sl]),
            "kpd": np.ascontiguousarray(kpd[sl]),
            "vpd": np.ascontiguousarray(vpd[sl]),
            "maskT": np.ascontiguousarray(mT[sl]),
        })
    return maps


def _gather(results):
    outA = np.concatenate([results[c]["outA"] for c in range(NCORES)], axis=0)
    outB = np.concatenate([results[c]["outB"] for c in range(NCORES)], axis=0)
    # outA: [BH, NSC, 65, SC]: rows 0-63 pv-even, row 64 den-lo
    # outB: [BH, NSC, 65, SC]: row 0 den-hi, rows 1-64 pv-odd
    num = outA[:, :, 0:64, :] + outB[:, :, 1:65, :]
    den = outA[:, :, 64, :] + outB[:, :, 0, :]
    out = num / (KEEP * den[:, :, None, :])
    return (out.transpose(0, 1, 3, 2).reshape(B, H, S, E)
            .astype(np.float32, copy=False))


def kernel(**inputs):
    nc = _build()
    maps = _in_maps(inputs)
    res = bass_utils.run_bass_kernel_spmd(nc, maps, core_ids=list(range(NCORES)))
    return _gather(res.results)


if __name__ == "__main__":
    _build()
    print("build+compile OK")


# revision 27
# speedup vs baseline: 1.0068x; 1.0068x over previous
"""Trainium2 Bass kernel for nn_AttentionModel (B=4,S=2048,H=8,E=64, dropout mask).

Sharding: 32 (b,h) pairs over 8 cores (4 pairs/core). Device computes, per
(pair, s-chunk-of-1024) unit, transposed-score attention with ALL main-loop
matmuls in one 64x64 PE-tiling config so the four quadrant tiles can run
concurrently (no mode-switch drains):

  step u (= t-rows 128u..128u+128 of one s-chunk of 1024):
    scores: 4 quadrant MMs K=64(e) M=64(t) N=512 -> sp[128,1024] F32 psum
            bank0 (s 0:512)   <- row-0 tiles (0,0)+(0,64)
            bank1 (s 512:1024)<- row-64 tiles (64,0)+(64,64)  [q/k dup'd]
    exp:    one ACT instr [128,1024] (scores pre-scaled by 1/8 on host)
    mask:   DMA [128,1024] fp16; pr = ex*mk on DVE (fp16 2x mode)
    PV/den: per s-half, 4 concurrent quadrant MMs:
            PV-even (0,0) -> pvA[0:64], den-lo (0,64) ones[64,64] -> pvA[64:128]
            PV-odd (64,64) -> pvB[64:128], den-hi (64,0) -> pvB[0:64]
  finalize: DVE copy psum->SBUF, DMA unnormalized PV + den rows to DRAM.

Host does the QKV projections (BLAS), all transposes/dup-layout prep, and the
final (pvA+pvB)/(0.9*den) normalization + gather.
"""

import sys

sys.path.insert(0, "/opt/trn_rl_repo")

import numpy as np

import concourse.bass as bass
import concourse.mybir as mybir
import concourse.tile as tile
from concourse import bacc, bass_utils
from concourse.bass import ds, ts

B, S, H, E = 4, 2048, 8, 64
NCORES = 8
PAIRS = (B * H) // NCORES  # 4 pairs per core
SC = 1024                  # s-chunk width
NSC = S // SC              # 2
NTT = S // 128             # 16 t-tiles (steps) per unit
DEPTH = 4                  # PV trails scores by DEPTH steps
MPF = 6                    # mask DMA prefetch distance (steps)
F32 = mybir.dt.float32
FP16 = mybir.dt.float16
KEEP = 0.9

_CACHED_NC = None


def _body(tc, qpd_d, kpd_d, vpd_d, mT_d, outA_d, outB_d):
    nc = tc.nc
    Exp = mybir.ActivationFunctionType.Exp
    with (
        tc.tile_pool(name="const", bufs=1) as const,
        tc.tile_pool(name="io", bufs=2) as io,
        tc.tile_pool(name="mk", bufs=MPF + 2) as mkp,
        tc.tile_pool(name="work", bufs=11) as work,
        tc.tile_pool(name="fin", bufs=2) as fin,
        tc.tile_pool(name="psS", bufs=2, space=bass.MemorySpace.PSUM) as psS,
        tc.tile_pool(name="psA", bufs=1, space=bass.MemorySpace.PSUM) as psA,
        tc.tile_pool(name="psB", bufs=1, space=bass.MemorySpace.PSUM) as psB,
    ):
        onesw = const.tile([128, 64], FP16, tag="onesw")
        nc.vector.memset(onesw[:, :], 1.0)

        # per-pair input tiles (double-buffered across pairs)
        def load_pair(p, eng):
            qpd = io.tile([128, S], FP16, tag="qpd", name="qpd")
            kpd = io.tile([128, S], FP16, tag="kpd", name="kpd")
            vpd = io.tile([128, NTT * E], FP16, tag="vpd", name="vpd")
            eng.dma_start(qpd[:, :], qpd_d[p])
            eng.dma_start(kpd[:, :], kpd_d[p])
            eng.dma_start(vpd[:, :], vpd_d[p])
            return qpd, kpd, vpd

        # SWDGE warmup: pay the Q7 first-use cost off the critical path
        warm = const.tile([128, 1], FP16, tag="warm")
        nc.gpsimd.dma_start(warm[:, :], qpd_d[0, :, 0:1])
        pair_tiles = {}

        units = [(p, c) for p in range(PAIRS) for c in range(NSC)]
        N = len(units) * NTT  # 128 steps
        exs, prs, pvts, mks = {}, {}, {}, {}

        def load_mask(gj, eng=None):
            unit, u = divmod(gj, NTT)
            p, c = units[unit]
            mk = mkp.tile([128, SC], FP16, tag="mk", name="mk")
            (eng or nc.sync).dma_start(
                mk[:, :], mT_d[p, ds(128 * u, 128), ds(c * SC, SC)])
            mks[gj] = mk

        def scores_step(gj):
            unit, u = divmod(gj, NTT)
            p, c = units[unit]
            if c == 0 and u == 0 and p + 1 < PAIRS:
                pair_tiles[p + 1] = load_pair(p + 1, nc.gpsimd)
            if gj + MPF < N:
                load_mask(gj + MPF)
            qpd, kpd, vpd = pair_tiles[p]
            sp = psS.tile([128, SC], F32, tag="sp", name="sp")
            t0 = 128 * u
            # 4 concurrent quadrant MMs; row-0 tiles -> bank0, row-64 -> bank1
            nc.tensor.matmul(sp[0:64, 0:512], kpd[0:64, ds(t0, 64)],
                             qpd[0:64, ds(c * SC, 512)],
                             start=True, stop=True, tile_position=(0, 0))
            nc.tensor.matmul(sp[64:128, 0:512], kpd[0:64, ds(t0 + 64, 64)],
                             qpd[0:64, ds(c * SC, 512)],
                             start=True, stop=True, tile_position=(0, 64))
            nc.tensor.matmul(sp[0:64, 512:1024], kpd[64:128, ds(t0, 64)],
                             qpd[64:128, ds(c * SC + 512, 512)],
                             start=True, stop=True, tile_position=(64, 0))
            nc.tensor.matmul(sp[64:128, 512:1024],
                             kpd[64:128, ds(t0 + 64, 64)],
                             qpd[64:128, ds(c * SC + 512, 512)],
                             start=True, stop=True, tile_position=(64, 64))
            ex = work.tile([128, SC], FP16, tag="ex", name="ex")
            nc.scalar.activation(ex[:, :], sp[:, :], Exp)
            exs[gj] = ex

        def mul_step(gj):
            ex = exs[gj]
            mk = mks.pop(gj)
            pr = work.tile([128, SC], FP16, tag="pr", name="pr")
            nc.vector.tensor_mul(pr[:, :], ex[:, :], mk[:, :])
            prs[gj] = pr

        def pv_step(gj):
            unit, u = divmod(gj, NTT)
            p, c = units[unit]
            _, _, vpd = pair_tiles[p]
            ex, pr = exs.pop(gj), prs.pop(gj)
            if u == 0:
                pvA = psA.tile([128, SC], F32, tag="pvA", name="pvA")
                pvB = psB.tile([128, SC], F32, tag="pvB", name="pvB")
                pvts[unit] = (pvA, pvB)
            pvA, pvB = pvts[unit]
            st = (u == 0)
            sp_ = (u == NTT - 1)
            vsl = ts(u, E)
            for s in range(2):      # s-half (512 cols)
                o = ds(s * 512, 512)
                # PV first: its start=True clears the bank before the
                # start=False den MMs write into it
                nc.tensor.matmul(pvA[0:64, o], vpd[0:64, vsl],
                                 pr[0:64, o], start=st, stop=sp_,
                                 tile_position=(0, 0))
                nc.tensor.matmul(pvB[64:128, o], vpd[64:128, vsl],
                                 pr[64:128, o], start=st, stop=sp_,
                                 tile_position=(64, 64))
                nc.tensor.matmul(pvA[64:128, o], onesw[0:64, :],
                                 ex[0:64, o], start=st, stop=sp_,
                                 tile_position=(0, 64))
                nc.tensor.matmul(pvB[0:64, o], onesw[64:128, :],
                                 ex[64:128, o], start=st, stop=sp_,
                                 tile_position=(64, 0))

        def finalize(unit):
            p, c = units[unit]
            last = unit == len(units) - 1
            pvA, pvB = pvts.pop(unit)
            obA = fin.tile([128, SC], F32, tag="obA", name="obA")
            nc.vector.tensor_copy(obA[:, :], pvA[:, :])
            obB = fin.tile([128, SC], F32, tag="obB", name="obB")
            if last:
                # exps are done; ScalarE copy runs parallel to the DVE one
                nc.scalar.copy(obB[:, :], pvB[:, :])
            else:
                nc.vector.tensor_copy(obB[:, :], pvB[:, :])
            engA = nc.sync if last else nc.gpsimd
            engB = nc.gpsimd if last else nc.gpsimd
            engA.dma_start(outA_d[p, c, 0:64], obA[0:64, :])
            engA.dma_start(outA_d[p, c, 64:65], obA[64:65, :])
            engB.dma_start(outB_d[p, c, 0:1], obB[0:1, :])
            engB.dma_start(outB_d[p, c, 1:65], obB[64:128, :])

        # scores(idx) issued first each iteration (keeps ACT fed), PV at a
        # uniform lag of DEPTH, finalize immediately after a unit's last PV
        # (its evac copies enter the DVE queue ahead of the next mul).
        # pv(gj) at uniform lag DEPTH, except a unit's LAST pv step is
        # co-issued one iteration early (with its second-to-last), so the
        # evac gets 2 iterations of runway before the next unit's PV chain
        # needs the psum banks.
        # prologue: q/k for pair 0 first (scores(0) needs them), then the
        # first masks, then v (not needed until the first pv step)
        qpd0 = io.tile([128, S], FP16, tag="qpd", name="qpd")
        kpd0 = io.tile([128, S], FP16, tag="kpd", name="kpd")
        vpd0 = io.tile([128, NTT * E], FP16, tag="vpd", name="vpd")
        nc.sync.dma_start(qpd0[:, :], qpd_d[0])
        nc.gpsimd.dma_start(kpd0[:, :], kpd_d[0])
        for g in range(MPF):
            load_mask(g, nc.gpsimd)
        nc.sync.dma_start(vpd0[:, :], vpd_d[0])
        pair_tiles[0] = (qpd0, kpd0, vpd0)
        for idx in range(N + DEPTH):
            gj = idx - DEPTH
            boundary = gj >= 0 and (gj + 2) % NTT == NTT - 1
            if idx < N:
                scores_step(idx)
                if not boundary:
                    mul_step(idx)
            if gj >= 0 and gj % NTT < NTT - 2:
                pv_step(gj)
                if boundary:
                    pv_step(gj + 1)      # last two pv steps of the unit,
                    pv_step(gj + 2)      # co-issued two iterations early
                    finalize(gj // NTT)
                    if idx < N:
                        mul_step(idx)


def _build():
    global _CACHED_NC
    if _CACHED_NC is not None:
        return _CACHED_NC
    nc = bacc.Bacc("TRN2", target_bir_lowering=False, debug=False,
                   num_devices=NCORES)
    qpd_d = nc.dram_tensor("qpd", [PAIRS, 128, S], FP16,
                           kind="ExternalInput").ap()
    kpd_d = nc.dram_tensor("kpd", [PAIRS, 128, S], FP16,
                           kind="ExternalInput").ap()
    vpd_d = nc.dram_tensor("vpd", [PAIRS, 128, NTT * E], FP16,
                           kind="ExternalInput").ap()
    mT_d = nc.dram_tensor("maskT", [PAIRS, S, S], FP16,
                          kind="ExternalInput").ap()
    outA_d = nc.dram_tensor("outA", [PAIRS, NSC, 65, SC], F32,
                            kind="ExternalOutput").ap()
    outB_d = nc.dram_tensor("outB", [PAIRS, NSC, 65, SC], F32,
                            kind="ExternalOutput").ap()
    with tile.TileContext(nc) as tc:
        _body(tc, qpd_d, kpd_d, vpd_d, mT_d, outA_d, outB_d)
    nc.compile()
    _CACHED_NC = nc
    return nc


def _in_maps(inputs):
    f32 = np.float32
    query = np.asarray(inputs["query"], f32)
    key = np.asarray(inputs["key"], f32)
    value = np.asarray(inputs["value"], f32)
    mask = np.asarray(inputs["drop_mask"])
    Wq, bq = np.asarray(inputs["Wq"], f32), np.asarray(inputs["bq"], f32)
    Wk, bk = np.asarray(inputs["Wk"], f32), np.asarray(inputs["bk"], f32)
    Wv, bv = np.asarray(inputs["Wv"], f32), np.asarray(inputs["bv"], f32)

    # host-side projections (BLAS) -- [B,S,H,E] @ [E,E] + b
    qp = (query.reshape(-1, E) @ Wq + bq).reshape(B, S, H, E)
    kp = (key.reshape(-1, E) @ Wk + bk).reshape(B, S, H, E)
    vp = (value.reshape(-1, E) @ Wv + bv).reshape(B, S, H, E)

    # qpd/kpd: [BH, E, S] fp16, duplicated across partition halves -> [BH,128,S]
    qpT = (qp.transpose(0, 2, 3, 1).reshape(B * H, E, S) * (1.0 / 8.0))
    kpT = kp.transpose(0, 2, 3, 1).reshape(B * H, E, S)
    qpd = np.concatenate([qpT, qpT], axis=1).astype(np.float16)
    kpd = np.concatenate([kpT, kpT], axis=1).astype(np.float16)
    # vpd: [BH, 128, 16*E]: partition p, block u holds v'[t=128u+p, :]
    vpd = (vp.transpose(0, 2, 1, 3).reshape(B * H, NTT, 128, E)
           .transpose(0, 2, 1, 3).reshape(B * H, 128, NTT * E)
           .astype(np.float16))
    # mask transposed [BH, t, s] as fp16 {0,1}
    mT = (np.ascontiguousarray(mask.transpose(0, 1, 3, 2))
          .astype(np.float16).reshape(B * H, S, S))

    maps = []
    for cidx in range(NCORES):
        sl = slice(cidx * PAIRS, (cidx + 1) * PAIRS)
        maps.append({
            "qpd": np.ascontiguousarray(qpd[sl]),
            "kpd": np.ascontiguousarray(kpd[sl]),
            "vpd": np.ascontiguousarray(vpd[sl]),
            "maskT": np.ascontiguousarray(mT[sl]),
        })
    return maps


def _gather(results):
    outA = np.concatenate([results[c]["outA"] for c in range(NCORES)], axis=0)
    outB = np.concatenate([results[c]["outB"] for c in range(NCORES)], axis=0)
    # outA: [BH, NSC, 65, SC]: rows 0-63 pv-even, row 64 den-lo
    # outB: [BH, NSC, 65, SC]: row 0 den-hi, rows 1-64 pv-odd
    num = outA[:, :, 0:64, :] + outB[:, :, 1:65, :]
    den = outA[:, :, 64, :] + outB[:, :, 0, :]
    out = num / (KEEP * den[:, :, None, :])
    return (out.transpose(0, 1, 3, 2).reshape(B, H, S, E)
            .astype(np.float32, copy=False))


def kernel(**inputs):
    nc = _build()
    maps = _in_maps(inputs)
    res = bass_utils.run_bass_kernel_spmd(nc, maps, core_ids=list(range(NCORES)))
    return _gather(res.results)


if __name__ == "__main__":
    _build()
    print("build+compile OK")


# revision 28
# speedup vs baseline: 1.0136x; 1.0068x over previous
"""Trainium2 Bass kernel for nn_AttentionModel (B=4,S=2048,H=8,E=64, dropout mask).

Sharding: 32 (b,h) pairs over 8 cores (4 pairs/core). Device computes, per
(pair, s-chunk-of-1024) unit, transposed-score attention with ALL main-loop
matmuls in one 64x64 PE-tiling config so the four quadrant tiles can run
concurrently (no mode-switch drains):

  step u (= t-rows 128u..128u+128 of one s-chunk of 1024):
    scores: 4 quadrant MMs K=64(e) M=64(t) N=512 -> sp[128,1024] F32 psum
            bank0 (s 0:512)   <- row-0 tiles (0,0)+(0,64)
            bank1 (s 512:1024)<- row-64 tiles (64,0)+(64,64)  [q/k dup'd]
    exp:    one ACT instr [128,1024] (scores pre-scaled by 1/8 on host)
    mask:   DMA [128,1024] fp16; pr = ex*mk on DVE (fp16 2x mode)
    PV/den: per s-half, 4 concurrent quadrant MMs:
            PV-even (0,0) -> pvA[0:64], den-lo (0,64) ones[64,64] -> pvA[64:128]
            PV-odd (64,64) -> pvB[64:128], den-hi (64,0) -> pvB[0:64]
  finalize: DVE copy psum->SBUF, DMA unnormalized PV + den rows to DRAM.

Host does the QKV projections (BLAS), all transposes/dup-layout prep, and the
final (pvA+pvB)/(0.9*den) normalization + gather.
"""

import sys

sys.path.insert(0, "/opt/trn_rl_repo")

import numpy as np

import concourse.bass as bass
import concourse.mybir as mybir
import concourse.tile as tile
from concourse import bacc, bass_utils
from concourse.bass import ds, ts

B, S, H, E = 4, 2048, 8, 64
NCORES = 8
PAIRS = (B * H) // NCORES  # 4 pairs per core
SC = 1024                  # s-chunk width
NSC = S // SC              # 2
NTT = S // 128             # 16 t-tiles (steps) per unit
DEPTH = 4                  # PV trails scores by DEPTH steps
MPF = 6                    # mask DMA prefetch distance (steps)
F32 = mybir.dt.float32
FP16 = mybir.dt.float16
KEEP = 0.9

_CACHED_NC = None


def _body(tc, qpd_d, kpd_d, vpd_d, mT_d, outA_d, outB_d):
    nc = tc.nc
    Exp = mybir.ActivationFunctionType.Exp
    with (
        tc.tile_pool(name="const", bufs=1) as const,
        tc.tile_pool(name="io", bufs=2) as io,
        tc.tile_pool(name="mk", bufs=MPF + 2) as mkp,
        tc.tile_pool(name="work", bufs=11) as work,
        tc.tile_pool(name="fin", bufs=2) as fin,
        tc.tile_pool(name="psS", bufs=2, space=bass.MemorySpace.PSUM) as psS,
        tc.tile_pool(name="psA", bufs=1, space=bass.MemorySpace.PSUM) as psA,
        tc.tile_pool(name="psB", bufs=1, space=bass.MemorySpace.PSUM) as psB,
    ):
        onesw = const.tile([128, 64], FP16, tag="onesw")
        nc.vector.memset(onesw[:, :], 1.0)

        # per-pair input tiles (double-buffered across pairs)
        def load_pair(p, eng):
            qpd = io.tile([128, S], FP16, tag="qpd", name="qpd")
            kpd = io.tile([128, S], FP16, tag="kpd", name="kpd")
            vpd = io.tile([128, NTT * E], FP16, tag="vpd", name="vpd")
            eng.dma_start(qpd[:, :], qpd_d[p])
            eng.dma_start(kpd[:, :], kpd_d[p])
            eng.dma_start(vpd[:, :], vpd_d[p])
            return qpd, kpd, vpd

        # SWDGE warmup: pay the Q7 first-use cost off the critical path
        warm = const.tile([128, 1], FP16, tag="warm")
        nc.gpsimd.dma_start(warm[:, :], qpd_d[0, :, 0:1])
        pair_tiles = {}

        units = [(p, c) for p in range(PAIRS) for c in range(NSC)]
        N = len(units) * NTT  # 128 steps
        exs, prs, pvts, mks = {}, {}, {}, {}

        def load_mask(gj, eng=None):
            unit, u = divmod(gj, NTT)
            p, c = units[unit]
            mk = mkp.tile([128, SC], FP16, tag="mk", name="mk")
            (eng or nc.sync).dma_start(
                mk[:, :], mT_d[p, ds(128 * u, 128), ds(c * SC, SC)])
            mks[gj] = mk

        def scores_step(gj):
            unit, u = divmod(gj, NTT)
            p, c = units[unit]
            if c == 0 and u == 0 and p + 1 < PAIRS:
                pair_tiles[p + 1] = load_pair(p + 1, nc.gpsimd)
            if gj + MPF < N:
                load_mask(gj + MPF)
            qpd, kpd, vpd = pair_tiles[p]
            sp = psS.tile([128, SC], F32, tag="sp", name="sp")
            t0 = 128 * u
            # 4 concurrent quadrant MMs; row-0 tiles -> bank0, row-64 -> bank1
            nc.tensor.matmul(sp[0:64, 0:512], kpd[0:64, ds(t0, 64)],
                             qpd[0:64, ds(c * SC, 512)],
                             start=True, stop=True, tile_position=(0, 0))
            nc.tensor.matmul(sp[64:128, 0:512], kpd[0:64, ds(t0 + 64, 64)],
                             qpd[0:64, ds(c * SC, 512)],
                             start=True, stop=True, tile_position=(0, 64))
            nc.tensor.matmul(sp[0:64, 512:1024], kpd[64:128, ds(t0, 64)],
                             qpd[64:128, ds(c * SC + 512, 512)],
                             start=True, stop=True, tile_position=(64, 0))
            nc.tensor.matmul(sp[64:128, 512:1024],
                             kpd[64:128, ds(t0 + 64, 64)],
                             qpd[64:128, ds(c * SC + 512, 512)],
                             start=True, stop=True, tile_position=(64, 64))
            ex = work.tile([128, SC], FP16, tag="ex", name="ex")
            nc.scalar.activation(ex[:, :], sp[:, :], Exp)
            exs[gj] = ex

        def mul_step(gj):
            ex = exs[gj]
            mk = mks.pop(gj)
            pr = work.tile([128, SC], FP16, tag="pr", name="pr")
            nc.vector.tensor_mul(pr[:, :], ex[:, :], mk[:, :])
            prs[gj] = pr

        def pv_step(gj):
            unit, u = divmod(gj, NTT)
            p, c = units[unit]
            _, _, vpd = pair_tiles[p]
            ex, pr = exs.pop(gj), prs.pop(gj)
            if u == 0:
                pvA = psA.tile([128, SC], F32, tag="pvA", name="pvA")
                pvB = psB.tile([128, SC], F32, tag="pvB", name="pvB")
                pvts[unit] = (pvA, pvB)
            pvA, pvB = pvts[unit]
            st = (u == 0)
            sp_ = (u == NTT - 1)
            vsl = ts(u, E)
            for s in range(2):      # s-half (512 cols)
                o = ds(s * 512, 512)
                # PV first: its start=True clears the bank before the
                # start=False den MMs write into it
                nc.tensor.matmul(pvA[0:64, o], vpd[0:64, vsl],
                                 pr[0:64, o], start=st, stop=sp_,
                                 tile_position=(0, 0))
                nc.tensor.matmul(pvB[64:128, o], vpd[64:128, vsl],
                                 pr[64:128, o], start=st, stop=sp_,
                                 tile_position=(64, 64))
                nc.tensor.matmul(pvA[64:128, o], onesw[0:64, :],
                                 ex[0:64, o], start=st, stop=sp_,
                                 tile_position=(0, 64))
                nc.tensor.matmul(pvB[0:64, o], onesw[64:128, :],
                                 ex[64:128, o], start=st, stop=sp_,
                                 tile_position=(64, 0))

        def finalize(unit):
            p, c = units[unit]
            last = unit == len(units) - 1
            pvA, pvB = pvts.pop(unit)
            obA = fin.tile([128, SC], F32, tag="obA", name="obA")
            nc.vector.tensor_copy(obA[:, :], pvA[:, :])
            obB = fin.tile([128, SC], F32, tag="obB", name="obB")
            if last:
                # exps are done; ScalarE copy runs parallel to the DVE one
                nc.scalar.copy(obB[:, :], pvB[:, :])
            else:
                nc.vector.tensor_copy(obB[:, :], pvB[:, :])
            engA = nc.sync if last else nc.gpsimd
            engB = nc.gpsimd if last else nc.gpsimd
            engA.dma_start(outA_d[p, c, 0:64], obA[0:64, :])
            engA.dma_start(outA_d[p, c, 64:65], obA[64:65, :])
            engB.dma_start(outB_d[p, c, 0:1], obB[0:1, :])
            engB.dma_start(outB_d[p, c, 1:65], obB[64:128, :])

        # scores(idx) issued first each iteration (keeps ACT fed), PV at a
        # uniform lag of DEPTH, finalize immediately after a unit's last PV
        # (its evac copies enter the DVE queue ahead of the next mul).
        # pv(gj) at uniform lag DEPTH, except a unit's LAST pv step is
        # co-issued one iteration early (with its second-to-last), so the
        # evac gets 2 iterations of runway before the next unit's PV chain
        # needs the psum banks.
        # prologue: q/k for pair 0 first (scores(0) needs them), then the
        # first masks, then v (not needed until the first pv step)
        qpd0 = io.tile([128, S], FP16, tag="qpd", name="qpd")
        kpd0 = io.tile([128, S], FP16, tag="kpd", name="kpd")
        vpd0 = io.tile([128, NTT * E], FP16, tag="vpd", name="vpd")
        nc.sync.dma_start(qpd0[:, :], qpd_d[0])
        nc.sync.dma_start(kpd0[:, :], kpd_d[0])
        for g in range(MPF):
            load_mask(g, nc.gpsimd)
        nc.sync.dma_start(vpd0[:, :], vpd_d[0])
        pair_tiles[0] = (qpd0, kpd0, vpd0)
        for idx in range(N + DEPTH):
            gj = idx - DEPTH
            boundary = gj >= 0 and (gj + 2) % NTT == NTT - 1
            if idx < N:
                scores_step(idx)
                if not boundary:
                    mul_step(idx)
            if gj >= 0 and gj % NTT < NTT - 2:
                pv_step(gj)
                if boundary:
                    pv_step(gj + 1)      # last two pv steps of the unit,
                    pv_step(gj + 2)      # co-issued two iterations early
                    finalize(gj // NTT)
                    if idx < N:
                        mul_step(idx)


def _build():
    global _CACHED_NC
    if _CACHED_NC is not None:
        return _CACHED_NC
    nc = bacc.Bacc("TRN2", target_bir_lowering=False, debug=False,
                   num_devices=NCORES)
    qpd_d = nc.dram_tensor("qpd", [PAIRS, 128, S], FP16,
                           kind="ExternalInput").ap()
    kpd_d = nc.dram_tensor("kpd", [PAIRS, 128, S], FP16,
                           kind="ExternalInput").ap()
    vpd_d = nc.dram_tensor("vpd", [PAIRS, 128, NTT * E], FP16,
                           kind="ExternalInput").ap()
    mT_d = nc.dram_tensor("maskT", [PAIRS, S, S], FP16,
                          kind="ExternalInput").ap()
    outA_d = nc.dram_tensor("outA", [PAIRS, NSC, 65, SC], F32,
                            kind="ExternalOutput").ap()
    outB_d = nc.dram_tensor("outB", [PAIRS, NSC, 65, SC], F32,
                            kind="ExternalOutput").ap()
    with tile.TileContext(nc) as tc:
        _body(tc, qpd_d, kpd_d, vpd_d, mT_d, outA_d, outB_d)
    nc.compile()
    _CACHED_NC = nc
    return nc


def _in_maps(inputs):
    f32 = np.float32
    query = np.asarray(inputs["query"], f32)
    key = np.asarray(inputs["key"], f32)
    value = np.asarray(inputs["value"], f32)
    mask = np.asarray(inputs["drop_mask"])
    Wq, bq = np.asarray(inputs["Wq"], f32), np.asarray(inputs["bq"], f32)
    Wk, bk = np.asarray(inputs["Wk"], f32), np.asarray(inputs["bk"], f32)
    Wv, bv = np.asarray(inputs["Wv"], f32), np.asarray(inputs["bv"], f32)

    # host-side projections (BLAS) -- [B,S,H,E] @ [E,E] + b
    qp = (query.reshape(-1, E) @ Wq + bq).reshape(B, S, H, E)
    kp = (key.reshape(-1, E) @ Wk + bk).reshape(B, S, H, E)
    vp = (value.reshape(-1, E) @ Wv + bv).reshape(B, S, H, E)

    # qpd/kpd: [BH, E, S] fp16, duplicated across partition halves -> [BH,128,S]
    qpT = (qp.transpose(0, 2, 3, 1).reshape(B * H, E, S) * (1.0 / 8.0))
    kpT = kp.transpose(0, 2, 3, 1).reshape(B * H, E, S)
    qpd = np.concatenate([qpT, qpT], axis=1).astype(np.float16)
    kpd = np.concatenate([kpT, kpT], axis=1).astype(np.float16)
    # vpd: [BH, 128, 16*E]: partition p, block u holds v'[t=128u+p, :]
    vpd = (vp.transpose(0, 2, 1, 3).reshape(B * H, NTT, 128, E)
           .transpose(0, 2, 1, 3).reshape(B * H, 128, NTT * E)
           .astype(np.float16))
    # mask transposed [BH, t, s] as fp16 {0,1}
    mT = (np.ascontiguousarray(mask.transpose(0, 1, 3, 2))
          .astype(np.float16).reshape(B * H, S, S))

    maps = []
    for cidx in range(NCORES):
        sl = slice(cidx * PAIRS, (cidx + 1) * PAIRS)
        maps.append({
            "qpd": np.ascontiguousarray(qpd[sl]),
            "kpd": np.ascontiguousarray(kpd[sl]),
            "vpd": np.ascontiguousarray(vpd[sl]),
            "maskT": np.ascontiguousarray(mT[sl]),
        })
    return maps


def _gather(results):
    outA = np.concatenate([results[c]["outA"] for c in range(NCORES)], axis=0)
    outB = np.concatenate([results[c]["outB"] for c in range(NCORES)], axis=0)
    # outA: [BH, NSC, 65, SC]: rows 0-63 pv-even, row 64 den-lo
    # outB: [BH, NSC, 65, SC]: row 0 den-hi, rows 1-64 pv-odd
    num = outA[:, :, 0:64, :] + outB[:, :, 1:65, :]
    den = outA[:, :, 64, :] + outB[:, :, 0, :]
    out = num / (KEEP * den[:, :, None, :])
    return (out.transpose(0, 1, 3, 2).reshape(B, H, S, E)
            .astype(np.float32, copy=False))


def kernel(**inputs):
    nc = _build()
    maps = _in_maps(inputs)
    res = bass_utils.run_bass_kernel_spmd(nc, maps, core_ids=list(range(NCORES)))
    return _gather(res.results)


if __name__ == "__main__":
    _build()
    print("build+compile OK")


# revision 29
# speedup vs baseline: 1.0357x; 1.0218x over previous
"""Trainium2 Bass kernel for nn_AttentionModel (B=4,S=2048,H=8,E=64, dropout mask).

Sharding: 32 (b,h) pairs over 8 cores (4 pairs/core). Device computes, per
(pair, s-chunk-of-1024) unit, transposed-score attention with ALL main-loop
matmuls in one 64x64 PE-tiling config so the four quadrant tiles can run
concurrently (no mode-switch drains):

  step u (= t-rows 128u..128u+128 of one s-chunk of 1024):
    scores: 4 quadrant MMs K=64(e) M=64(t) N=512 -> sp[128,1024] F32 psum
            bank0 (s 0:512)   <- row-0 tiles (0,0)+(0,64)
            bank1 (s 512:1024)<- row-64 tiles (64,0)+(64,64)  [q/k dup'd]
    exp:    one ACT instr [128,1024] (scores pre-scaled by 1/8 on host)
    mask:   DMA [128,1024] fp16; pr = ex*mk on DVE (fp16 2x mode)
    PV/den: per s-half, 4 concurrent quadrant MMs:
            PV-even (0,0) -> pvA[0:64], den-lo (0,64) ones[64,64] -> pvA[64:128]
            PV-odd (64,64) -> pvB[64:128], den-hi (64,0) -> pvB[0:64]
  finalize: DVE copy psum->SBUF, DMA unnormalized PV + den rows to DRAM.

Host does the QKV projections (BLAS), all transposes/dup-layout prep, and the
final (pvA+pvB)/(0.9*den) normalization + gather.
"""

import sys

sys.path.insert(0, "/opt/trn_rl_repo")

import numpy as np

import concourse.bass as bass
import concourse.mybir as mybir
import concourse.tile as tile
from concourse import bacc, bass_utils
from concourse.bass import ds, ts

B, S, H, E = 4, 2048, 8, 64
NCORES = 8
PAIRS = (B * H) // NCORES  # 4 pairs per core
SC = 1024                  # s-chunk width
NSC = S // SC              # 2
NTT = S // 128             # 16 t-tiles (steps) per unit
DEPTH = 4                  # PV trails scores by DEPTH steps
MPF = 6                    # mask DMA prefetch distance (steps)
F32 = mybir.dt.float32
FP16 = mybir.dt.float16
KEEP = 0.9

_CACHED_NC = None


def _body(tc, qpd_d, kpd_d, vpd_d, mT_d, outA_d, outB_d):
    nc = tc.nc
    Exp = mybir.ActivationFunctionType.Exp
    with (
        tc.tile_pool(name="const", bufs=1) as const,
        tc.tile_pool(name="io", bufs=2) as io,
        tc.tile_pool(name="mk", bufs=MPF + 2) as mkp,
        tc.tile_pool(name="work", bufs=11) as work,
        tc.tile_pool(name="fin", bufs=2) as fin,
        tc.tile_pool(name="psS", bufs=2, space=bass.MemorySpace.PSUM) as psS,
        tc.tile_pool(name="psA", bufs=1, space=bass.MemorySpace.PSUM) as psA,
        tc.tile_pool(name="psB", bufs=1, space=bass.MemorySpace.PSUM) as psB,
    ):
        onesw = const.tile([128, 64], FP16, tag="onesw")
        nc.vector.memset(onesw[:, :], 1.0)

        # per-pair input tiles (double-buffered across pairs)
        def load_pair(p, eng):
            qpd = io.tile([128, S], FP16, tag="qpd", name="qpd")
            kpd = io.tile([128, S], FP16, tag="kpd", name="kpd")
            vpd = io.tile([128, NTT * E], FP16, tag="vpd", name="vpd")
            eng.dma_start(qpd[:, :], qpd_d[p])
            eng.dma_start(kpd[:, :], kpd_d[p])
            eng.dma_start(vpd[:, :], vpd_d[p])
            return qpd, kpd, vpd

        # SWDGE warmup: pay the Q7 first-use cost off the critical path
        warm = const.tile([128, 1], FP16, tag="warm")
        nc.gpsimd.dma_start(warm[:, :], qpd_d[0, :, 0:1])
        pair_tiles = {}

        units = [(p, c) for p in range(PAIRS) for c in range(NSC)]
        N = len(units) * NTT  # 128 steps
        exs, prs, pvts, mks = {}, {}, {}, {}

        def load_mask(gj, eng=None):
            unit, u = divmod(gj, NTT)
            p, c = units[unit]
            mk = mkp.tile([128, SC], FP16, tag="mk", name="mk")
            (eng or nc.sync).dma_start(
                mk[:, :], mT_d[p, ds(128 * u, 128), ds(c * SC, SC)])
            mks[gj] = mk

        def scores_step(gj):
            unit, u = divmod(gj, NTT)
            p, c = units[unit]
            if c == 0 and u == 0 and p + 1 < PAIRS:
                pair_tiles[p + 1] = load_pair(p + 1, nc.gpsimd)
            if gj + MPF < N:
                load_mask(gj + MPF)
            qpd, kpd, vpd = pair_tiles[p]
            sp = psS.tile([128, SC], F32, tag="sp", name="sp")
            t0 = 128 * u
            # 4 concurrent quadrant MMs; row-0 tiles -> bank0, row-64 -> bank1
            nc.tensor.matmul(sp[0:64, 0:512], kpd[0:64, ds(t0, 64)],
                             qpd[0:64, ds(c * SC, 512)],
                             start=True, stop=True, tile_position=(0, 0))
            nc.tensor.matmul(sp[64:128, 0:512], kpd[0:64, ds(t0 + 64, 64)],
                             qpd[0:64, ds(c * SC, 512)],
                             start=True, stop=True, tile_position=(0, 64))
            nc.tensor.matmul(sp[0:64, 512:1024], kpd[64:128, ds(t0, 64)],
                             qpd[64:128, ds(c * SC + 512, 512)],
                             start=True, stop=True, tile_position=(64, 0))
            nc.tensor.matmul(sp[64:128, 512:1024],
                             kpd[64:128, ds(t0 + 64, 64)],
                             qpd[64:128, ds(c * SC + 512, 512)],
                             start=True, stop=True, tile_position=(64, 64))
            ex = work.tile([128, SC], FP16, tag="ex", name="ex")
            nc.scalar.activation(ex[:, :], sp[:, :], Exp)
            exs[gj] = ex

        def mul_step(gj):
            ex = exs[gj]
            mk = mks.pop(gj)
            pr = work.tile([128, SC], FP16, tag="pr", name="pr")
            nc.vector.tensor_mul(pr[:, :], ex[:, :], mk[:, :])
            prs[gj] = pr

        def pv_step(gj):
            unit, u = divmod(gj, NTT)
            p, c = units[unit]
            _, _, vpd = pair_tiles[p]
            ex, pr = exs.pop(gj), prs.pop(gj)
            if u == 0:
                pvA = psA.tile([128, SC], F32, tag="pvA", name="pvA")
                pvB = psB.tile([128, SC], F32, tag="pvB", name="pvB")
                pvts[unit] = (pvA, pvB)
            pvA, pvB = pvts[unit]
            st = (u == 0)
            sp_ = (u == NTT - 1)
            vsl = ts(u, E)
            for s in range(2):      # s-half (512 cols)
                o = ds(s * 512, 512)
                # PV first: its start=True clears the bank before the
                # start=False den MMs write into it
                nc.tensor.matmul(pvA[0:64, o], vpd[0:64, vsl],
                                 pr[0:64, o], start=st, stop=sp_,
                                 tile_position=(0, 0))
                nc.tensor.matmul(pvB[64:128, o], vpd[64:128, vsl],
                                 pr[64:128, o], start=st, stop=sp_,
                                 tile_position=(64, 64))
                nc.tensor.matmul(pvA[64:128, o], onesw[0:64, :],
                                 ex[0:64, o], start=st, stop=sp_,
                                 tile_position=(0, 64))
                nc.tensor.matmul(pvB[0:64, o], onesw[64:128, :],
                                 ex[64:128, o], start=st, stop=sp_,
                                 tile_position=(64, 0))

        def finalize(unit):
            p, c = units[unit]
            pvA, pvB = pvts.pop(unit)
            obA = fin.tile([128, SC], F32, tag="obA", name="obA")
            nc.vector.tensor_copy(obA[:, :], pvA[:, :])
            obB = fin.tile([128, SC], F32, tag="obB", name="obB")
            nc.vector.tensor_copy(obB[:, :], pvB[:, :])
            eng = nc.sync if unit == len(units) - 1 else nc.gpsimd
            eng.dma_start(outA_d[p, c, 0:64], obA[0:64, :])
            eng.dma_start(outA_d[p, c, 64:65], obA[64:65, :])
            eng.dma_start(outB_d[p, c, 0:1], obB[0:1, :])
            eng.dma_start(outB_d[p, c, 1:65], obB[64:128, :])

        # scores(idx) issued first each iteration (keeps ACT fed), PV at a
        # uniform lag of DEPTH, finalize immediately after a unit's last PV
        # (its evac copies enter the DVE queue ahead of the next mul).
        # pv(gj) at uniform lag DEPTH, except a unit's LAST pv step is
        # co-issued one iteration early (with its second-to-last), so the
        # evac gets 2 iterations of runway before the next unit's PV chain
        # needs the psum banks.
        # prologue: q/k for pair 0 first (scores(0) needs them), then the
        # first masks, then v (not needed until the first pv step)
        qpd0 = io.tile([128, S], FP16, tag="qpd", name="qpd")
        kpd0 = io.tile([128, S], FP16, tag="kpd", name="kpd")
        vpd0 = io.tile([128, NTT * E], FP16, tag="vpd", name="vpd")
        nc.sync.dma_start(qpd0[:, :], qpd_d[0])
        nc.sync.dma_start(kpd0[:, :], kpd_d[0])
        for g in range(MPF):
            load_mask(g, nc.gpsimd)
        nc.sync.dma_start(vpd0[:, :], vpd_d[0])
        pair_tiles[0] = (qpd0, kpd0, vpd0)
        for idx in range(N + DEPTH):
            gj = idx - DEPTH
            boundary = gj >= 0 and (gj + 2) % NTT == NTT - 1
            if idx < N:
                scores_step(idx)
                if not boundary:
                    mul_step(idx)
            if gj >= 0 and gj % NTT < NTT - 2:
                pv_step(gj)
                if boundary:
                    pv_step(gj + 1)      # last two pv steps of the unit,
                    pv_step(gj + 2)      # co-issued two iterations early
                    finalize(gj // NTT)
                    if idx < N:
                        mul_step(idx)


def _build():
    global _CACHED_NC
    if _CACHED_NC is not None:
        return _CACHED_NC
    nc = bacc.Bacc("TRN2", target_bir_lowering=False, debug=False,
                   num_devices=NCORES)
    qpd_d = nc.dram_tensor("qpd", [PAIRS, 128, S], FP16,
                           kind="ExternalInput").ap()
    kpd_d = nc.dram_tensor("kpd", [PAIRS, 128, S], FP16,
                           kind="ExternalInput").ap()
    vpd_d = nc.dram_tensor("vpd", [PAIRS, 128, NTT * E], FP16,
                           kind="ExternalInput").ap()
    mT_d = nc.dram_tensor("maskT", [PAIRS, S, S], FP16,
                          kind="ExternalInput").ap()
    outA_d = nc.dram_tensor("outA", [PAIRS, NSC, 65, SC], F32,
                            kind="ExternalOutput").ap()
    outB_d = nc.dram_tensor("outB", [PAIRS, NSC, 65, SC], F32,
                            kind="ExternalOutput").ap()
    with tile.TileContext(nc) as tc:
        _body(tc, qpd_d, kpd_d, vpd_d, mT_d, outA_d, outB_d)
    nc.compile()
    _CACHED_NC = nc
    return nc


def _in_maps(inputs):
    f32 = np.float32
    query = np.asarray(inputs["query"], f32)
    key = np.asarray(inputs["key"], f32)
    value = np.asarray(inputs["value"], f32)
    mask = np.asarray(inputs["drop_mask"])
    Wq, bq = np.asarray(inputs["Wq"], f32), np.asarray(inputs["bq"], f32)
    Wk, bk = np.asarray(inputs["Wk"], f32), np.asarray(inputs["bk"], f32)
    Wv, bv = np.asarray(inputs["Wv"], f32), np.asarray(inputs["bv"], f32)

    # host-side projections (BLAS) -- [B,S,H,E] @ [E,E] + b
    qp = (query.reshape(-1, E) @ Wq + bq).reshape(B, S, H, E)
    kp = (key.reshape(-1, E) @ Wk + bk).reshape(B, S, H, E)
    vp = (value.reshape(-1, E) @ Wv + bv).reshape(B, S, H, E)

    # qpd/kpd: [BH, E, S] fp16, duplicated across partition halves -> [BH,128,S]
    qpT = (qp.transpose(0, 2, 3, 1).reshape(B * H, E, S) * (1.0 / 8.0))
    kpT = kp.transpose(0, 2, 3, 1).reshape(B * H, E, S)
    qpd = np.concatenate([qpT, qpT], axis=1).astype(np.float16)
    kpd = np.concatenate([kpT, kpT], axis=1).astype(np.float16)
    # vpd: [BH, 128, 16*E]: partition p, block u holds v'[t=128u+p, :]
    vpd = (vp.transpose(0, 2, 1, 3).reshape(B * H, NTT, 128, E)
           .transpose(0, 2, 1, 3).reshape(B * H, 128, NTT * E)
           .astype(np.float16))
    # mask transposed [BH, t, s] as fp16 {0,1}
    mT = (np.ascontiguousarray(mask.transpose(0, 1, 3, 2))
          .astype(np.float16).reshape(B * H, S, S))

    maps = []
    for cidx in range(NCORES):
        sl = slice(cidx * PAIRS, (cidx + 1) * PAIRS)
        maps.append({
            "qpd": np.ascontiguousarray(qpd[sl]),
            "kpd": np.ascontiguousarray(kpd[sl]),
            "vpd": np.ascontiguousarray(vpd[sl]),
            "maskT": np.ascontiguousarray(mT[sl]),
        })
    return maps


def _gather(results):
    outA = np.concatenate([results[c]["outA"] for c in range(NCORES)], axis=0)
    outB = np.concatenate([results[c]["outB"] for c in range(NCORES)], axis=0)
    # outA: [BH, NSC, 65, SC]: rows 0-63 pv-even, row 64 den-lo
    # outB: [BH, NSC, 65, SC]: row 0 den-hi, rows 1-64 pv-odd
    num = outA[:, :, 0:64, :] + outB[:, :, 1:65, :]
    den = outA[:, :, 64, :] + outB[:, :, 0, :]
    out = num / (KEEP * den[:, :, None, :])
    return (out.transpose(0, 1, 3, 2).reshape(B, H, S, E)
            .astype(np.float32, copy=False))


def kernel(**inputs):
    nc = _build()
    maps = _in_maps(inputs)
    res = bass_utils.run_bass_kernel_spmd(nc, maps, core_ids=list(range(NCORES)))
    return _gather(res.results)


if __name__ == "__main__":
    _build()
    print("build+compile OK")
